# revision 28
# baseline (speedup 1.0000x reference)
"""Trainium2 Bass kernel for nn_Map_79748952752358 (dense_cnn).

Pipeline: LSTM encoder (40 steps) -> e2d projection -> big linear
(lin1: 256 -> 262144) -> per-sample dynamic 1x1 conv over feature
[1024, 32x32] -> BN(eval) -> channel-max -> clip.

Sharding (v2):
  - LSTM + e2d replicated on all 8 cores (serial recurrence, tiny state).
  - lin1 tensor-sharded over R: core k computes filter rows r in
    [32k, 32k+32) for ALL 16 samples (1/8 of the 134MB W1 streams per
    core; prefetched into SBUF during the LSTM so lin1 is PE-bound).
  - One AllToAll redistributes filters so core k holds the FULL
    [256, 1024] filter block for ITS two samples (2k, 2k+1).
  - conv batch-sharded: each core convolves its 2 samples with full
    R=256 (M=128 matmuls, feature slice only 4.2MB/core).
  - per-core output: channel-max partials [128, 64]; host combines the
    four 32-row groups, applies the BN-shift floor T0 and the clip.

Key math folds (exact, host side):
  - BN scale s=gamma/sqrt(var+eps) > 0 folded into lin1 weights/bias.
  - relu(x)+t maxed over r == max(max_r(x+t), max_r(t)); +t injected
    into conv PSUM via a rank-1 matmul, max_r(t)=T0 applied on host.
"""

import numpy as np
import ml_dtypes

import concourse.bass as bass
import concourse.mybir as mybir
from concourse import tile
from concourse.tile import ScopedClock
from concourse.alu_op_type import AluOpType
from concourse.bass_utils import run_bass_kernel_spmd

BF16 = ml_dtypes.bfloat16

B, S, V, E, HID = 16, 40, 1004, 256, 256
C, R, HW2 = 1024, 256, 1024
BN_EPS = 1e-5
N_CORES = 8
RS = R // N_CORES   # 32 r-rows per core (lin1 shard)
BS = B // N_CORES   # 2 samples per core (conv shard)
P = 128

AFT = mybir.ActivationFunctionType
AX = mybir.AxisListType


# ---------------------------------------------------------------------------
# Tile tail-drain patch: this walrus build accepts fewer sem waits per
# TPB_CTRL instruction than Tile's exit drain accumulates; split them into
# single-wait SP nops.
_drain_patched = False


def _patch_tile_drain():
    global _drain_patched
    if _drain_patched:
        return
    _drain_patched = True

    def _patched(self, tick_clock, wait_clock):
        nc = self.nc
        probe = nc.sync.nop(nofuse=True, hint="drain_wait_split")
        wait_clock.add_sem_waits(
            probe.ins, ScopedClock({None: tick_clock.global_clock})
        )
        si = probe.ins.sync_info
        waits = list(si.on_wait or []) if si is not None else []
        if len(waits) > 1:
            si.on_wait = waits[:1]
            for w in waits[1:]:
                n = nc.sync.nop(nofuse=True, hint="drain_wait_split")
                nsi = n.ins.sync_info
                if nsi is None:
                    import bass_rust

                    n.ins.sync_info = bass_rust.SyncInfo(on_wait=[w], on_update=[])
                else:
                    nsi.on_wait = [w]
        nc.sync.drain()
        nc.all_engine_barrier()
        assert self.sems is not None
        popped = nc._tile_sem_poison_stack.pop()
        assert popped is self._sem_poison
        nc.clear_and_free_semaphores(list(self.sems.allocated().values()))
        nc.all_engine_barrier()

    tile.TileContext._drain_and_barrier = _patched


_ws_counter = [0]


def _split_excess_waits(nc, limit=1):
    """Walrus on this image rejects instructions with more than ~2 sem waits.
    Move excess waits onto same-engine EventSemaphore carriers inserted just
    before the offending instruction (same per-engine stream order, identical
    blocking semantics)."""
    import bass_rust

    for fn in nc.m.functions:
        for bb in fn.blocks:
            out = []
            for inst in bb.instructions:
                si = inst.sync_info
                waits = list(si.on_wait or []) if si is not None else []
                if len(waits) > limit:
                    for w in waits[:-limit]:
                        _ws_counter[0] += 1
                        carrier = mybir.InstEventSemaphore(
                            name=f"I-waitsplit-{_ws_counter[0]}",
                            opcode="EventSemaphore",
                            engine=inst.engine,
                            sync_info=bass_rust.SyncInfo(
                                on_wait=[w], on_update=[]),
                        )
                        out.append(carrier)
                    si.on_wait = waits[-limit:]
                out.append(inst)
            bb.instructions = out


# ---------------------------------------------------------------------------
def _build_program(slots):
    """Build the SPMD Bass program. `slots[b]` = length[b]-1, the LSTM step
    whose hidden state is each sample's final state (compile-time constants).
    """
    _patch_tile_drain()
    nc = bass.Bass("TRN2", target_bir_lowering=False, debug=False,
                   num_devices=N_CORES)
    dt = mybir.dt
    f32, bf16 = dt.float32, dt.bfloat16

    def din(name, shape, d=bf16):
        return nc.dram_tensor(name, shape, d, kind="ExternalInput").ap()

    # feature slice for this core's 2 samples: (b, c_in_chunk, kc, hw)
    feat_d = din("feat", [BS, P, 8, HW2])
    # one packed bf16 constant block: embT | wihT | whhT | e2dT | eye |
    # b1 | delta  (partition-padded blocks)
    PK = [2 * S * B, 16 * P, 16 * P, 4 * P, P, 8 * P, RS * B]
    pack_d = din("cpack", [P, sum(PK)])
    biasf_d = din("biasf", [P, 12], f32)   # biasg(8) | e2db(2) | tvec(2)
    w1_d = din("w1T", [P, 8 * RS * 2 * P])        # tiles (ct, r, kh)

    out_d = nc.dram_tensor("part_out", [P, BS * 2 * 2 * 16], f32,
                           kind="ExternalOutput").ap()

    with tile.TileContext(nc) as tc:
        with (
            tc.tile_pool(name="const", bufs=1) as cpool,
            tc.tile_pool(name="xg", bufs=1) as xgpool,
            tc.tile_pool(name="hist", bufs=1) as hpool,
            tc.tile_pool(name="gs", bufs=2) as gspool,
            tc.tile_pool(name="cell", bufs=1) as cellpool,
            tc.tile_pool(name="tmp", bufs=4) as tmppool,
            tc.tile_pool(name="w1c", bufs=6) as w1pool,
            tc.tile_pool(name="f1", bufs=1) as f1pool,
            tc.tile_pool(name="feat", bufs=2) as fpool,
            tc.tile_pool(name="f1c", bufs=2) as f1cpool,
            tc.tile_pool(name="rmax", bufs=4) as rmpool,
            tc.tile_pool(name="vout", bufs=1) as vpool,
            tc.tile_pool(name="a2a", bufs=1, space="DRAM") as dpool,
        ):
            pack = cpool.tile([P, sum(PK)], bf16, tag="cpack")
            nc.sync.dma_start(out=pack[:], in_=pack_d)
            biasf = cpool.tile([P, 12], f32, tag="biasf")
            nc.sync.dma_start(out=biasf[:], in_=biasf_d)

            off = np.cumsum([0] + PK)
            embT = pack[:, off[0]:off[1]]
            wih = pack[:, off[1]:off[2]]
            whh = pack[:, off[2]:off[3]]
            e2dT = pack[:, off[3]:off[4]]
            eye = pack[:, off[4]:off[5]]
            b1 = pack[0:RS, off[5]:off[6]]
            delta = pack[0:RS, off[6]:off[7]]
            biasg = biasf[:, 0:8]
            e2db = biasf[:, 8:10]
            tvec = biasf[:, 10:12]                # BN shift per conv r-tile

            # ---- prefetched loads: w1 chunks (6 of 8 resident), feature ----
            CW = RS * 2 * P  # w1 chunk cols per ct

            def w1_fetch(ct):
                wch = w1pool.tile([P, CW], bf16, tag="w1c", name=f"wch{ct}")
                nc.sync.dma_start(out=wch[:], in_=w1_d[:, ct * CW:(ct + 1) * CW])
                return wch

            wchs = []
            for ct in range(6):
                wchs.append(w1_fetch(ct))
                if ct == 3:
                    # feature slice DMA sits between w1 chunk 3 and 4 so it
                    # is in flight well before the conv needs it.
                    fbs = []
                    for b in range(BS):
                        fb = fpool.tile([P, 8 * HW2], bf16, tag="feat",
                                        name=f"fb{b}")
                        nc.sync.dma_start(
                            out=fb[:].rearrange("p (kc hw) -> p kc hw", kc=8),
                            in_=feat_d[b])
                        fbs.append(fb)

            # ---- Stage A: xg = w_ih @ x_t for all steps (+ gate bias) ----
            xg_s = xgpool.tile([P, 8 * S * B], bf16)
            NCH = 320  # psum N-chunk: 20 steps x 16
            with tc.tile_pool(name="xpsum", bufs=2, space="PSUM") as xpsum:
                for n in range(2):       # n-outer: steps 0-19 ready first
                    for m in range(8):
                        ps = xpsum.tile([P, NCH], f32, tag="xg")
                        for ke in range(2):
                            nc.tensor.matmul(
                                ps[:],
                                lhsT=wih[:, (ke * 8 + m) * P:(ke * 8 + m + 1) * P],
                                rhs=embT[:, ke * S * B + n * NCH: ke * S * B + (n + 1) * NCH],
                                start=(ke == 0), stop=(ke == 1),
                            )
                        nc.scalar.activation(
                            out=xg_s[:, m * S * B + n * NCH: m * S * B + (n + 1) * NCH],
                            in_=ps[:], func=AFT.Identity, bias=biasg[:, m:m + 1],
                        )

            # ---- Stage B: LSTM recurrence (layout: gate-dim on partitions) --
            S_eff = max(slots) + 1
            hist = hpool.tile([P, S * 2 * B], bf16)   # (t, kh, b)
            c_s = cellpool.tile([P, 2 * B], f32)      # (kh, b)
            xg_r = xg_s[:].rearrange("p (m t b) -> p m t b", m=8, t=S)
            # final-h capture target (filled inside the loop at t==slots[b])
            h_fin = cellpool.tile([P, 2 * B], bf16, tag="hfin")  # (kh, b)
            hf_r = h_fin[:].rearrange("p (k b) -> p b k", k=2)
            lstm_psum = tc.tile_pool(name="gpsum", bufs=2, space="PSUM")
            gpsum = lstm_psum.__enter__()
            for t in range(S_eff):
                gp = gpsum.tile([P, P], f32, tag="gates")
                nc.tensor.matmul(gp[:], lhsT=eye[:], rhs=xg_r[:, :, t, :],
                                 start=True, stop=(t == 0))
                if t > 0:
                    for m in range(8):
                        for kh in range(2):
                            nc.tensor.matmul(
                                gp[:, m * B:(m + 1) * B],
                                lhsT=whh[:, (kh * 8 + m) * P:(kh * 8 + m + 1) * P],
                                rhs=hist[:, (t - 1) * 2 * B + kh * B:
                                         (t - 1) * 2 * B + (kh + 1) * B],
                                start=False, stop=(m == 7 and kh == 1),
                                skip_group_check=True,
                            )
                gs = gspool.tile([P, P], f32, tag="gs")
                # cols (m,b): i=0:32, f=32:64, g=64:96, o=96:128
                # one sigmoid for all gates; tanh(g)=2*sig(2g)-1 (g-rows
                # pre-scaled by 2 on host; fp32 to avoid rounding blowup)
                nc.scalar.activation(out=gs[:], in_=gp[:], func=AFT.Sigmoid)
                tg = tmppool.tile([P, 2 * B], f32, tag="tg")
                nc.vector.tensor_scalar(tg[:], gs[:, 64:96], 2.0, -1.0,
                                        AluOpType.mult, AluOpType.add)
                t1 = tmppool.tile([P, 2 * B], f32, tag="t1")
                nc.vector.tensor_tensor(t1[:], gs[:, 0:32], tg[:],
                                        AluOpType.mult)
                if t == 0:
                    nc.vector.tensor_copy(c_s[:], t1[:])
                else:
                    t2 = tmppool.tile([P, 2 * B], f32, tag="t2")
                    nc.vector.tensor_tensor(t2[:], gs[:, 32:64], c_s[:],
                                            AluOpType.mult)
                    nc.vector.tensor_tensor(c_s[:], t1[:], t2[:], AluOpType.add)
                th = tmppool.tile([P, 2 * B], bf16, tag="th")
                nc.scalar.activation(out=th[:], in_=c_s[:], func=AFT.Tanh)
                nc.vector.tensor_tensor(
                    hist[:, t * 2 * B:(t + 1) * 2 * B],
                    gs[:, 96:128], th[:], AluOpType.mult)
                # capture final h for samples ending at this step (hides in
                # the per-step DVE slack)
                src = hist[:, t * 2 * B:(t + 1) * 2 * B]
                for b in range(B):
                    if slots[b] == t:
                        nc.vector.tensor_copy(
                            hf_r[:, b],
                            src.rearrange("p (k b) -> p b k", k=2)[:, b])

            # ---- e2d projection: instrT = tanh(e2d_w @ h + b) -------------
            instrT = cellpool.tile([P, 2 * B], bf16, tag="instrT")  # (kh, b)
            for m in range(2):
                pe2 = gpsum.tile([P, B], f32, tag="e2d")
                for kh in range(2):
                    nc.tensor.matmul(
                        pe2[:],
                        lhsT=e2dT[:, (kh * 2 + m) * P:(kh * 2 + m + 1) * P],
                        rhs=h_fin[:, kh * B:(kh + 1) * B],
                        start=(kh == 0), stop=(kh == 1),
                    )
                nc.scalar.activation(out=instrT[:, m * B:(m + 1) * B],
                                     in_=pe2[:], func=AFT.Tanh,
                                     bias=e2db[:, m:m + 1])
            lstm_psum.__exit__(None, None, None)

            # ---- lin1 (c-chunk slice): core k computes f1[all r, c-chunk k]
            # for all 16 samples.  PSUM partitions = c_local; loop over 8
            # r-tiles of 32.  f1_sb cols = b*256 + r  (b-major so the
            # per-dest a2a slices are contiguous).
            # The exchange is split in two r-halves (= conv r-tiles m):
            # half 0's AllToAll overlaps lin1's second half; conv on half 0
            # overlaps the second AllToAll.
            f1_sb = f1pool.tile([P, B * R], bf16)
            a2a_in = [dpool.tile([N_CORES * P, BS * P], bf16,
                                 name=f"a2ain{h}") for h in range(2)]
            a2a_out = [dpool.tile([N_CORES * P, BS * P], bf16,
                                  name=f"a2aout{h}") for h in range(2)]
            f1v = f1_sb[:].rearrange("p (b r) -> p r b", b=B)
            f1h = f1_sb[:].rearrange("p (b h r) -> p h b r", b=B, h=2)

            def a2a_send(h):
                # shard j of half h: [c_local, (b2)(r128)] from f1_sb
                iv = a2a_in[h][:].rearrange("(j c) x -> j c x", j=N_CORES)
                for j in range(N_CORES):
                    nc.sync.dma_start(
                        out=iv[j].rearrange("c (b r) -> c b r", b=BS),
                        in_=f1h[:, h, 2 * j:2 * j + 2])

            lin1_psum = tc.tile_pool(name="lpsum", bufs=4, space="PSUM")
            lpsum = lin1_psum.__enter__()
            for rt in range(8):
                wch = wchs[rt]
                if rt + 6 < 8:
                    wchs.append(w1_fetch(rt + 6))
                pb = lpsum.tile([P, RS * B], f32, tag="lin1")
                nc.tensor.matmul(pb[:], lhsT=b1[:, rt * P:(rt + 1) * P],
                                 rhs=delta[:], start=True, stop=False,
                                 skip_group_check=True)
                for r in range(RS):
                    for kh in range(2):
                        nc.tensor.matmul(
                            pb[:, r * B:(r + 1) * B],
                            lhsT=wch[:, (r * 2 + kh) * P:(r * 2 + kh + 1) * P],
                            rhs=instrT[:, kh * B:(kh + 1) * B],
                            start=False, stop=(r == RS - 1 and kh == 1),
                            skip_group_check=True,
                        )
                # pb cols (r, b) -> f1_sb strided (b, r-slice rt)
                nc.scalar.activation(out=f1v[:, rt * RS:(rt + 1) * RS],
                                     in_=pb[:], func=AFT.Lrelu, alpha=0.01)
                if rt == 3:
                    a2a_send(0)
                    nc.gpsimd.collective_compute(
                        "AllToAll", AluOpType.bypass,
                        replica_groups=[list(range(N_CORES))],
                        ins=[a2a_in[0][:].opt()], outs=[a2a_out[0][:].opt()],
                    )
            lin1_psum.__exit__(None, None, None)
            a2a_send(1)
            nc.gpsimd.collective_compute(
                "AllToAll", AluOpType.bypass,
                replica_groups=[list(range(N_CORES))],
                ins=[a2a_in[1][:].opt()], outs=[a2a_out[1][:].opt()],
            )

            # ---- conv + fused BN-shift + channel max ----------------------
            # Per half m: gather shard kc (= c-chunk kc of my 2 samples,
            # 128 r) and immediately accumulate it into the 4 (b, n) PSUM
            # tiles; consumer DMAs pipeline with the matmuls.  BN shift
            # enters via the evacuation ACT bias (per-partition tvec).
            vout = vpool.tile([P, BS * 2 * 2 * 16], f32)  # [(j,q),(b,n,m,blk)]
            NH = HW2 // 2  # 512
            conv_psum = tc.tile_pool(name="cpsum", bufs=8, space="PSUM")
            cpsum = conv_psum.__enter__()
            for m in range(2):
                f1c = f1cpool.tile([P, BS * 8 * P], bf16, tag="f1c",
                                   name=f"f1c{m}")
                f1cv = f1c[:].rearrange("p (b kc r) -> p kc b r",
                                        b=BS, kc=8)
                f1t = f1c[:].rearrange("p (b kc r) -> p b kc r",
                                       b=BS, kc=8)
                ov = a2a_out[m][:].rearrange("(kc c) x -> kc c x", kc=N_CORES)
                pcs = {}
                for kc in range(8):
                    nc.sync.dma_start(
                        out=f1cv[:, kc],
                        in_=ov[kc].rearrange("c (b r) -> c b r", b=BS))
                    for b in range(BS):
                        for n in range(2):
                            if kc == 0:
                                pcs[(b, n)] = cpsum.tile(
                                    [P, NH], f32, tag="conv",
                                    name=f"pc{m}{b}{n}")
                            nc.tensor.matmul(
                                pcs[(b, n)][:],
                                lhsT=f1t[:, b, kc],
                                rhs=fbs[b][:, kc * HW2 + n * NH:
                                           kc * HW2 + (n + 1) * NH],
                                start=(kc == 0), stop=(kc == 7),
                                skip_group_check=True,
                            )
                for b in range(BS):
                    for n in range(2):
                        cp = rmpool.tile([P, NH], f32, tag="convcp")
                        nc.scalar.activation(out=cp[:], in_=pcs[(b, n)][:],
                                             func=AFT.Identity,
                                             bias=tvec[:, m:m + 1])
                        col = ((b * 2 + n) * 2 + m) * 16
                        nc.vector.tensor_reduce(
                            out=vout[:, col:col + 16],
                            in_=cp[:].rearrange("p (blk q) -> p blk q", q=32),
                            axis=AX.X, op=AluOpType.max, apply_transpose=True)
            conv_psum.__exit__(None, None, None)

            # contiguous store; host decodes the (j,q),(b,n,blk) layout
            nc.sync.dma_start(out=out_d, in_=vout[:])

    _split_excess_waits(nc)
    return nc


# ---------------------------------------------------------------------------
def _prep_inputs(feature, instruction_idx, instruction_length, emb_table,
                 w_ih, w_hh, b_ih, b_hh, e2d_w, e2d_b,
                 lin1_w, lin1_b, bn_gamma, bn_beta, bn_mean, bn_var):
    """Host-side layout/dtype prep. Returns (in_maps, slots, T0)."""
    f32 = np.float32

    def to_bf(x):
        return np.ascontiguousarray(x.astype(BF16))

    feature = np.asarray(feature, f32)
    emb_table = np.asarray(emb_table, f32)
    idx = np.asarray(instruction_idx)
    lengths = np.asarray(instruction_length).astype(np.int64)
    slots = [int(max(l, 1) - 1) for l in lengths]

    # feature (b, c_in, kc, hw): per-partition data contiguous (16KB)
    feat = to_bf(feature.reshape(B, 8, P, HW2).transpose(0, 2, 1, 3))

    # embeds transposed: [p, (ke, t*b)]
    emb = emb_table[idx]                       # [B, S, E]
    embT = emb.transpose(2, 1, 0).reshape(2, P, S * B)
    embT = to_bf(embT.transpose(1, 0, 2).reshape(P, 2 * S * B))

    def wtiles(w, kt, mt):
        # w: [out, in] -> lhsT tiles arr[p, (k, m, col)] with lhsT=w.T tile
        wt = np.asarray(w, f32).T  # [in, out]
        a = wt.reshape(kt, P, mt, P).transpose(1, 0, 2, 3)
        return to_bf(a.reshape(P, kt * mt * P))

    # tanh(g) computed as 2*sigmoid(2g)-1: scale the g-gate rows (512:768)
    # by 2 so one big sigmoid covers all four gates.
    gsc = np.ones((4 * HID, 1), f32)
    gsc[2 * HID:3 * HID] = 2.0
    wihT = wtiles(np.asarray(w_ih, f32) * gsc, 2, 8)
    whhT = wtiles(np.asarray(w_hh, f32) * gsc, 2, 8)
    e2dT = wtiles(e2d_w, 2, 2)

    bg = ((np.asarray(b_ih, f32) + np.asarray(b_hh, f32)) * gsc[:, 0]) \
        .reshape(8, P).T.copy()
    e2db = np.asarray(e2d_b, f32).reshape(2, P).T.copy()

    s = np.asarray(bn_gamma, f32) / np.sqrt(np.asarray(bn_var, f32) + BN_EPS)
    tsh = np.asarray(bn_beta, f32) - np.asarray(bn_mean, f32) * s
    T0 = float(tsh.max())

    w1s = np.asarray(lin1_w, f32).reshape(R, C, HID) * s[:, None, None]
    b1s = np.asarray(lin1_b, f32).reshape(R, C) * s[:, None]

    delta = np.repeat(np.eye(RS, dtype=f32), B, axis=1)  # [32, 512]
    eye = np.eye(P, dtype=f32)

    def pad128(a):
        out = np.zeros((P, a.shape[1]), f32)
        out[:a.shape[0]] = a
        return out

    # biasf: gate biases | e2d bias | BN-shift per conv r-tile (tvec)
    tvec = tsh.reshape(2, P).T.copy()
    biasf = np.concatenate([bg, e2db, tvec], axis=1).astype(f32)  # [128, 12]
    biasf = np.ascontiguousarray(biasf)

    in_maps = []
    for k in range(N_CORES):
        csl = slice(k * P, (k + 1) * P)
        wsl = w1s[:, csl]                       # [256, 128, 256] (r, c, h)
        # tiles (rt, r_local, kh): arr[p, ...] = W'T[kh*128+p, rt*32+rl, c]
        ws = wsl.transpose(2, 0, 1)             # [h, r, c]
        a = (ws.reshape(2, P, 8, RS, P)         # [kh, p, rt, rl, c]
             .transpose(1, 2, 3, 0, 4)          # [p, rt, rl, kh, c]
             .reshape(P, 8 * RS * 2 * P))
        # b1 inject tile per rt: [rl, (rt, c)]
        b1c = (b1s[:, csl].reshape(8, RS, P)    # [rt, rl, c]
               .transpose(1, 0, 2).reshape(RS, 8 * P))
        cpack = np.concatenate(
            [embT.astype(f32), wihT.astype(f32), whhT.astype(f32),
             e2dT.astype(f32), eye, pad128(b1c), pad128(delta)], axis=1)
        in_maps.append(dict(feat=feat[2 * k:2 * k + 2], cpack=to_bf(cpack),
                            biasf=biasf, w1T=to_bf(a)))
    return in_maps, slots, T0


_cache = {}


def _run(inputs, trace=False):
    (in_maps, slots, T0) = _prep_inputs(
        inputs["feature"], inputs["instruction_idx"],
        inputs["instruction_length"], inputs["emb_table"],
        inputs["w_ih"], inputs["w_hh"], inputs["b_ih"], inputs["b_hh"],
        inputs["e2d_w"], inputs["e2d_b"], inputs["lin1_w"], inputs["lin1_b"],
        inputs["bn_gamma"], inputs["bn_beta"], inputs["bn_mean"],
        inputs["bn_var"])

    key = tuple(slots)
    if key not in _cache:
        _cache[key] = _build_program(slots)
    nc = _cache[key]

    kw = {}
    if trace:
        kw = dict(trace=True, trace_cores=list(range(N_CORES)))
    res = run_bass_kernel_spmd(nc, in_maps, list(range(N_CORES)), **kw)
    # per-core out: [128=(j,q), 128=(b,n,m,blk)]; sample = 2*core + b,
    # hw = n*512 + blk*32 + q, value = max over 32-row group j of r-tile m.
    parts = np.stack([np.asarray(res.results[i]["part_out"], np.float32)
                      for i in range(N_CORES)])      # [8, 128, 128]
    v = parts.reshape(N_CORES, 4, 32, BS, 2, 2, 16)  # [c, j, q, b, n, m, blk]
    v = v.max(axis=(1, 5))                           # [core, q, b, n, blk]
    v = v.transpose(0, 2, 3, 4, 1)                   # [core, b, n, blk, q]
    single = v.reshape(B, HW2)
    single = np.maximum(single, T0)
    out = np.clip(single, 0.0, 1.0).reshape(B, 32, 32).astype(np.float32)
    return out, res


def kernel(**inputs) -> np.ndarray:
    out, _ = _run(inputs, trace=False)
    return out


def kernel_traced(**inputs):
    out, res = _run(inputs, trace=True)
    return out, res


# revision 32
# speedup vs baseline: 1.0699x; 1.0699x over previous
"""Trainium2 Bass kernel for nn_Map_79748952752358 (dense_cnn).

Pipeline: LSTM encoder (40 steps) -> e2d projection -> big linear
(lin1: 256 -> 262144) -> per-sample dynamic 1x1 conv over feature
[1024, 32x32] -> BN(eval) -> channel-max -> clip.

Sharding (v2):
  - LSTM + e2d replicated on all 8 cores (serial recurrence, tiny state).
  - lin1 tensor-sharded over R: core k computes filter rows r in
    [32k, 32k+32) for ALL 16 samples (1/8 of the 134MB W1 streams per
    core; prefetched into SBUF during the LSTM so lin1 is PE-bound).
  - One AllToAll redistributes filters so core k holds the FULL
    [256, 1024] filter block for ITS two samples (2k, 2k+1).
  - conv batch-sharded: each core convolves its 2 samples with full
    R=256 (M=128 matmuls, feature slice only 4.2MB/core).
  - per-core output: channel-max partials [128, 64]; host combines the
    four 32-row groups, applies the BN-shift floor T0 and the clip.

Key math folds (exact, host side):
  - BN scale s=gamma/sqrt(var+eps) > 0 folded into lin1 weights/bias.
  - relu(x)+t maxed over r == max(max_r(x+t), max_r(t)); +t injected
    into conv PSUM via a rank-1 matmul, max_r(t)=T0 applied on host.
"""

import numpy as np
import ml_dtypes

import concourse.bass as bass
import concourse.mybir as mybir
from concourse import tile
from concourse.tile import ScopedClock
from concourse.alu_op_type import AluOpType
from concourse.bass_utils import run_bass_kernel_spmd

BF16 = ml_dtypes.bfloat16

B, S, V, E, HID = 16, 40, 1004, 256, 256
C, R, HW2 = 1024, 256, 1024
BN_EPS = 1e-5
N_CORES = 8
RS = R // N_CORES   # 32 r-rows per core (lin1 shard)
BS = B // N_CORES   # 2 samples per core (conv shard)
P = 128

AFT = mybir.ActivationFunctionType
AX = mybir.AxisListType


# ---------------------------------------------------------------------------
# Tile tail-drain patch: this walrus build accepts fewer sem waits per
# TPB_CTRL instruction than Tile's exit drain accumulates; split them into
# single-wait SP nops.
_drain_patched = False


def _patch_tile_drain():
    global _drain_patched
    if _drain_patched:
        return
    _drain_patched = True

    def _patched(self, tick_clock, wait_clock):
        nc = self.nc
        probe = nc.sync.nop(nofuse=True, hint="drain_wait_split")
        wait_clock.add_sem_waits(
            probe.ins, ScopedClock({None: tick_clock.global_clock})
        )
        si = probe.ins.sync_info
        waits = list(si.on_wait or []) if si is not None else []
        if len(waits) > 1:
            si.on_wait = waits[:1]
            for w in waits[1:]:
                n = nc.sync.nop(nofuse=True, hint="drain_wait_split")
                nsi = n.ins.sync_info
                if nsi is None:
                    import bass_rust

                    n.ins.sync_info = bass_rust.SyncInfo(on_wait=[w], on_update=[])
                else:
                    nsi.on_wait = [w]
        nc.sync.drain()
        nc.all_engine_barrier()
        assert self.sems is not None
        popped = nc._tile_sem_poison_stack.pop()
        assert popped is self._sem_poison
        nc.clear_and_free_semaphores(list(self.sems.allocated().values()))
        nc.all_engine_barrier()

    tile.TileContext._drain_and_barrier = _patched


_ws_counter = [0]


def _split_excess_waits(nc, limit=1):
    """Walrus on this image rejects instructions with more than ~2 sem waits.
    Move excess waits onto same-engine EventSemaphore carriers inserted just
    before the offending instruction (same per-engine stream order, identical
    blocking semantics)."""
    import bass_rust

    for fn in nc.m.functions:
        for bb in fn.blocks:
            out = []
            for inst in bb.instructions:
                si = inst.sync_info
                waits = list(si.on_wait or []) if si is not None else []
                if len(waits) > limit:
                    for w in waits[:-limit]:
                        _ws_counter[0] += 1
                        carrier = mybir.InstEventSemaphore(
                            name=f"I-waitsplit-{_ws_counter[0]}",
                            opcode="EventSemaphore",
                            engine=inst.engine,
                            sync_info=bass_rust.SyncInfo(
                                on_wait=[w], on_update=[]),
                        )
                        out.append(carrier)
                    si.on_wait = waits[-limit:]
                out.append(inst)
            bb.instructions = out


# ---------------------------------------------------------------------------
def _build_program(slots):
    """Build the SPMD Bass program. `slots[b]` = length[b]-1, the LSTM step
    whose hidden state is each sample's final state (compile-time constants).
    """
    _patch_tile_drain()
    nc = bass.Bass("TRN2", target_bir_lowering=False, debug=False,
                   num_devices=N_CORES)
    dt = mybir.dt
    f32, bf16 = dt.float32, dt.bfloat16

    def din(name, shape, d=bf16):
        return nc.dram_tensor(name, shape, d, kind="ExternalInput").ap()

    # feature slice for this core's 2 samples: (b, c_in_chunk, kc, hw)
    feat_d = din("feat", [BS, P, 8, HW2])
    # one packed bf16 constant block: embT | wihT | whhT | e2dT | eye |
    # b1 | delta  (partition-padded blocks)
    PK = [2 * S * B, 16 * P, 16 * P, 4 * P, P, 8 * P, RS * B]
    pack_d = din("cpack", [P, sum(PK)])
    biasf_d = din("biasf", [P, 12], f32)   # biasg(8) | e2db(2) | tvec(2)
    w1_d = din("w1T", [P, 8 * RS * 2 * P])        # tiles (ct, r, kh)

    out_d = nc.dram_tensor("part_out", [P, BS * 2 * 2 * 16], f32,
                           kind="ExternalOutput").ap()

    with tile.TileContext(nc) as tc:
        with (
            tc.tile_pool(name="const", bufs=1) as cpool,
            tc.tile_pool(name="xg", bufs=1) as xgpool,
            tc.tile_pool(name="hist", bufs=1) as hpool,
            tc.tile_pool(name="gs", bufs=2) as gspool,
            tc.tile_pool(name="cell", bufs=1) as cellpool,
            tc.tile_pool(name="tmp", bufs=4) as tmppool,
            tc.tile_pool(name="w1c", bufs=6) as w1pool,
            tc.tile_pool(name="f1", bufs=1) as f1pool,
            tc.tile_pool(name="feat", bufs=2) as fpool,
            tc.tile_pool(name="f1c", bufs=2) as f1cpool,
            tc.tile_pool(name="rmax", bufs=4) as rmpool,
            tc.tile_pool(name="vout", bufs=1) as vpool,
            tc.tile_pool(name="a2a", bufs=1, space="DRAM") as dpool,
        ):
            pack = cpool.tile([P, sum(PK)], bf16, tag="cpack")
            nc.sync.dma_start(out=pack[:], in_=pack_d)
            biasf = cpool.tile([P, 12], f32, tag="biasf")
            nc.sync.dma_start(out=biasf[:], in_=biasf_d)

            off = np.cumsum([0] + PK)
            embT = pack[:, off[0]:off[1]]
            wih = pack[:, off[1]:off[2]]
            whh = pack[:, off[2]:off[3]]
            e2dT = pack[:, off[3]:off[4]]
            eye = pack[:, off[4]:off[5]]
            b1 = pack[0:RS, off[5]:off[6]]
            delta = pack[0:RS, off[6]:off[7]]
            biasg = biasf[:, 0:8]
            e2db = biasf[:, 8:10]
            tvec = biasf[:, 10:12]                # BN shift per conv r-tile

            # ---- prefetched loads: w1 chunks (6 of 8 resident), feature ----
            CW = RS * 2 * P  # w1 chunk cols per ct

            def w1_fetch(ct):
                wch = w1pool.tile([P, CW], bf16, tag="w1c", name=f"wch{ct}")
                nc.sync.dma_start(out=wch[:], in_=w1_d[:, ct * CW:(ct + 1) * CW])
                return wch

            wchs = []
            for ct in range(6):
                wchs.append(w1_fetch(ct))
                if ct == 3:
                    # feature slice DMA sits between w1 chunk 3 and 4 so it
                    # is in flight well before the conv needs it.
                    fbs = []
                    for b in range(BS):
                        fb = fpool.tile([P, 8 * HW2], bf16, tag="feat",
                                        name=f"fb{b}")
                        nc.sync.dma_start(
                            out=fb[:].rearrange("p (kc hw) -> p kc hw", kc=8),
                            in_=feat_d[b])
                        fbs.append(fb)

            # ---- Stage A: xg = w_ih @ x_t for all steps (+ gate bias) ----
            xg_s = xgpool.tile([P, 8 * S * B], bf16)
            NCH = 320  # psum N-chunk: 20 steps x 16
            with tc.tile_pool(name="xpsum", bufs=2, space="PSUM") as xpsum:
                for n in range(2):       # n-outer: steps 0-19 ready first
                    for m in range(8):
                        ps = xpsum.tile([P, NCH], f32, tag="xg")
                        for ke in range(2):
                            nc.tensor.matmul(
                                ps[:],
                                lhsT=wih[:, (ke * 8 + m) * P:(ke * 8 + m + 1) * P],
                                rhs=embT[:, ke * S * B + n * NCH: ke * S * B + (n + 1) * NCH],
                                start=(ke == 0), stop=(ke == 1),
                            )
                        nc.scalar.activation(
                            out=xg_s[:, m * S * B + n * NCH: m * S * B + (n + 1) * NCH],
                            in_=ps[:], func=AFT.Identity, bias=biasg[:, m:m + 1],
                        )

            # ---- Stage B: LSTM recurrence (layout: gate-dim on partitions) --
            S_eff = max(slots) + 1
            hist = hpool.tile([P, S * 2 * B], bf16)   # (t, kh, b)
            c_s = cellpool.tile([P, 2 * B], f32)      # (kh, b)
            xg_r = xg_s[:].rearrange("p (m t b) -> p m t b", m=8, t=S)
            # final-h capture target (filled inside the loop at t==slots[b])
            h_fin = cellpool.tile([P, 2 * B], bf16, tag="hfin")  # (kh, b)
            hf_r = h_fin[:].rearrange("p (k b) -> p b k", k=2)
            lstm_psum = tc.tile_pool(name="gpsum", bufs=2, space="PSUM")
            gpsum = lstm_psum.__enter__()
            for t in range(S_eff):
                gp = gpsum.tile([P, P], f32, tag="gates")
                nc.tensor.matmul(gp[:], lhsT=eye[:], rhs=xg_r[:, :, t, :],
                                 start=True, stop=(t == 0))
                if t > 0:
                    for m in range(8):
                        for kh in range(2):
                            nc.tensor.matmul(
                                gp[:, m * B:(m + 1) * B],
                                lhsT=whh[:, (kh * 8 + m) * P:(kh * 8 + m + 1) * P],
                                rhs=hist[:, (t - 1) * 2 * B + kh * B:
                                         (t - 1) * 2 * B + (kh + 1) * B],
                                start=False, stop=(m == 7 and kh == 1),
                                skip_group_check=True,
                            )
                gs = gspool.tile([P, P], bf16, tag="gs")
                # cols (m,b): i=0:32, f=32:64, g=64:96, o=96:128
                # one sigmoid for all gates; tanh(g)=2*sig(2g)-1 (g-rows
                # pre-scaled by 2 on host).  bf16 gates halve the ACT write
                # and enable the 2x DVE mode; the cell state stays fp32.
                nc.scalar.activation(out=gs[:], in_=gp[:], func=AFT.Sigmoid)
                tg = tmppool.tile([P, 2 * B], bf16, tag="tg")
                nc.vector.tensor_scalar(tg[:], gs[:, 64:96], 2.0, -1.0,
                                        AluOpType.mult, AluOpType.add)
                t1 = tmppool.tile([P, 2 * B], f32, tag="t1")
                nc.vector.tensor_tensor(t1[:], gs[:, 0:32], tg[:],
                                        AluOpType.mult)
                if t == 0:
                    nc.vector.tensor_copy(c_s[:], t1[:])
                else:
                    t2 = tmppool.tile([P, 2 * B], f32, tag="t2")
                    nc.vector.tensor_tensor(t2[:], gs[:, 32:64], c_s[:],
                                            AluOpType.mult)
                    nc.vector.tensor_tensor(c_s[:], t1[:], t2[:], AluOpType.add)
                th = tmppool.tile([P, 2 * B], bf16, tag="th")
                nc.scalar.activation(out=th[:], in_=c_s[:], func=AFT.Tanh)
                nc.vector.tensor_tensor(
                    hist[:, t * 2 * B:(t + 1) * 2 * B],
                    gs[:, 96:128], th[:], AluOpType.mult)
                # capture final h for samples ending at this step (hides in
                # the per-step DVE slack)
                src = hist[:, t * 2 * B:(t + 1) * 2 * B]
                for b in range(B):
                    if slots[b] == t:
                        nc.vector.tensor_copy(
                            hf_r[:, b],
                            src.rearrange("p (k b) -> p b k", k=2)[:, b])

            # ---- e2d projection: instrT = tanh(e2d_w @ h + b) -------------
            instrT = cellpool.tile([P, 2 * B], bf16, tag="instrT")  # (kh, b)
            for m in range(2):
                pe2 = gpsum.tile([P, B], f32, tag="e2d")
                for kh in range(2):
                    nc.tensor.matmul(
                        pe2[:],
                        lhsT=e2dT[:, (kh * 2 + m) * P:(kh * 2 + m + 1) * P],
                        rhs=h_fin[:, kh * B:(kh + 1) * B],
                        start=(kh == 0), stop=(kh == 1),
                    )
                nc.scalar.activation(out=instrT[:, m * B:(m + 1) * B],
                                     in_=pe2[:], func=AFT.Tanh,
                                     bias=e2db[:, m:m + 1])
            lstm_psum.__exit__(None, None, None)

            # ---- lin1 (c-chunk slice): core k computes f1[all r, c-chunk k]
            # for all 16 samples.  PSUM partitions = c_local; loop over 8
            # r-tiles of 32.  f1_sb cols = b*256 + r  (b-major so the
            # per-dest a2a slices are contiguous).
            f1_sb = f1pool.tile([P, B * R], bf16)
            a2a_in = dpool.tile([N_CORES * P, BS * R], bf16)
            a2a_out = dpool.tile([N_CORES * P, BS * R], bf16)
            f1v = f1_sb[:].rearrange("p (b r) -> p r b", b=B)
            lin1_psum = tc.tile_pool(name="lpsum", bufs=4, space="PSUM")
            lpsum = lin1_psum.__enter__()
            for rt in range(8):
                wch = wchs[rt]
                if rt + 6 < 8:
                    wchs.append(w1_fetch(rt + 6))
                pb = lpsum.tile([P, RS * B], f32, tag="lin1")
                nc.tensor.matmul(pb[:], lhsT=b1[:, rt * P:(rt + 1) * P],
                                 rhs=delta[:], start=True, stop=False,
                                 skip_group_check=True)
                for r in range(RS):
                    for kh in range(2):
                        nc.tensor.matmul(
                            pb[:, r * B:(r + 1) * B],
                            lhsT=wch[:, (r * 2 + kh) * P:(r * 2 + kh + 1) * P],
                            rhs=instrT[:, kh * B:(kh + 1) * B],
                            start=False, stop=(r == RS - 1 and kh == 1),
                            skip_group_check=True,
                        )
                # pb cols (r, b) -> f1_sb strided (b, r-slice rt)
                nc.scalar.activation(out=f1v[:, rt * RS:(rt + 1) * RS],
                                     in_=pb[:], func=AFT.Lrelu, alpha=0.01)
            lin1_psum.__exit__(None, None, None)

            # ---- AllToAll: ship my c-chunk of dest-core samples -----------
            a2a_iv = a2a_in[:].rearrange("(j c) x -> c j x", j=N_CORES)
            nc.sync.dma_start(
                out=a2a_iv,
                in_=f1_sb[:].rearrange("p (j x) -> p j x", j=N_CORES))
            nc.gpsimd.collective_compute(
                "AllToAll", AluOpType.bypass,
                replica_groups=[list(range(N_CORES))],
                ins=[a2a_in[:].opt()], outs=[a2a_out[:].opt()],
            )

            # ---- conv + fused BN-shift + channel max ----------------------
            # Per half m: gather shard kc (= c-chunk kc of my 2 samples,
            # 128 r) and immediately accumulate it into the 4 (b, n) PSUM
            # tiles; consumer DMAs pipeline with the matmuls.  BN shift
            # enters via the evacuation ACT bias (per-partition tvec).
            vout = vpool.tile([P, BS * 2 * 2 * 16], f32)  # [(j,q),(b,n,m,blk)]
            NH = HW2 // 2  # 512
            conv_psum = tc.tile_pool(name="cpsum", bufs=8, space="PSUM")
            cpsum = conv_psum.__enter__()
            for m in range(2):
                f1c = f1cpool.tile([P, BS * 8 * P], bf16, tag="f1c",
                                   name=f"f1c{m}")
                f1cv = f1c[:].rearrange("p (b kc r) -> p kc b r",
                                        b=BS, kc=8)
                f1t = f1c[:].rearrange("p (b kc r) -> p b kc r",
                                       b=BS, kc=8)
                ov = a2a_out[:].rearrange("(kc c) (b m2 r) -> kc m2 c b r",
                                          kc=N_CORES, b=BS, m2=2)
                pcs = {}
                for kc in range(8):
                    nc.sync.dma_start(out=f1cv[:, kc], in_=ov[kc, m])
                    for b in range(BS):
                        for n in range(2):
                            if kc == 0:
                                pcs[(b, n)] = cpsum.tile(
                                    [P, NH], f32, tag="conv",
                                    name=f"pc{m}{b}{n}")
                            nc.tensor.matmul(
                                pcs[(b, n)][:],
                                lhsT=f1t[:, b, kc],
                                rhs=fbs[b][:, kc * HW2 + n * NH:
                                           kc * HW2 + (n + 1) * NH],
                                start=(kc == 0), stop=(kc == 7),
                                skip_group_check=True,
                            )
                for b in range(BS):
                    for n in range(2):
                        cp = rmpool.tile([P, NH], f32, tag="convcp")
                        nc.scalar.activation(out=cp[:], in_=pcs[(b, n)][:],
                                             func=AFT.Identity,
                                             bias=tvec[:, m:m + 1])
                        col = ((b * 2 + n) * 2 + m) * 16
                        nc.vector.tensor_reduce(
                            out=vout[:, col:col + 16],
                            in_=cp[:].rearrange("p (blk q) -> p blk q", q=32),
                            axis=AX.X, op=AluOpType.max, apply_transpose=True)
            conv_psum.__exit__(None, None, None)

            # contiguous store; host decodes the (j,q),(b,n,blk) layout
            nc.sync.dma_start(out=out_d, in_=vout[:])

    _split_excess_waits(nc)
    return nc


# ---------------------------------------------------------------------------
def _prep_inputs(feature, instruction_idx, instruction_length, emb_table,
                 w_ih, w_hh, b_ih, b_hh, e2d_w, e2d_b,
                 lin1_w, lin1_b, bn_gamma, bn_beta, bn_mean, bn_var):
    """Host-side layout/dtype prep. Returns (in_maps, slots, T0)."""
    f32 = np.float32

    def to_bf(x):
        return np.ascontiguousarray(x.astype(BF16))

    feature = np.asarray(feature, f32)
    emb_table = np.asarray(emb_table, f32)
    idx = np.asarray(instruction_idx)
    lengths = np.asarray(instruction_length).astype(np.int64)
    slots = [int(max(l, 1) - 1) for l in lengths]

    # feature (b, c_in, kc, hw): per-partition data contiguous (16KB)
    feat = to_bf(feature.reshape(B, 8, P, HW2).transpose(0, 2, 1, 3))

    # embeds transposed: [p, (ke, t*b)]
    emb = emb_table[idx]                       # [B, S, E]
    embT = emb.transpose(2, 1, 0).reshape(2, P, S * B)
    embT = to_bf(embT.transpose(1, 0, 2).reshape(P, 2 * S * B))

    def wtiles(w, kt, mt):
        # w: [out, in] -> lhsT tiles arr[p, (k, m, col)] with lhsT=w.T tile
        wt = np.asarray(w, f32).T  # [in, out]
        a = wt.reshape(kt, P, mt, P).transpose(1, 0, 2, 3)
        return to_bf(a.reshape(P, kt * mt * P))

    # tanh(g) computed as 2*sigmoid(2g)-1: scale the g-gate rows (512:768)
    # by 2 so one big sigmoid covers all four gates.
    gsc = np.ones((4 * HID, 1), f32)
    gsc[2 * HID:3 * HID] = 2.0
    wihT = wtiles(np.asarray(w_ih, f32) * gsc, 2, 8)
    whhT = wtiles(np.asarray(w_hh, f32) * gsc, 2, 8)
    e2dT = wtiles(e2d_w, 2, 2)

    bg = ((np.asarray(b_ih, f32) + np.asarray(b_hh, f32)) * gsc[:, 0]) \
        .reshape(8, P).T.copy()
    e2db = np.asarray(e2d_b, f32).reshape(2, P).T.copy()

    s = np.asarray(bn_gamma, f32) / np.sqrt(np.asarray(bn_var, f32) + BN_EPS)
    tsh = np.asarray(bn_beta, f32) - np.asarray(bn_mean, f32) * s
    T0 = float(tsh.max())

    w1s = np.asarray(lin1_w, f32).reshape(R, C, HID) * s[:, None, None]
    b1s = np.asarray(lin1_b, f32).reshape(R, C) * s[:, None]

    delta = np.repeat(np.eye(RS, dtype=f32), B, axis=1)  # [32, 512]
    eye = np.eye(P, dtype=f32)

    def pad128(a):
        out = np.zeros((P, a.shape[1]), f32)
        out[:a.shape[0]] = a
        return out

    # biasf: gate biases | e2d bias | BN-shift per conv r-tile (tvec)
    tvec = tsh.reshape(2, P).T.copy()
    biasf = np.concatenate([bg, e2db, tvec], axis=1).astype(f32)  # [128, 12]
    biasf = np.ascontiguousarray(biasf)

    in_maps = []
    for k in range(N_CORES):
        csl = slice(k * P, (k + 1) * P)
        wsl = w1s[:, csl]                       # [256, 128, 256] (r, c, h)
        # tiles (rt, r_local, kh): arr[p, ...] = W'T[kh*128+p, rt*32+rl, c]
        ws = wsl.transpose(2, 0, 1)             # [h, r, c]
        a = (ws.reshape(2, P, 8, RS, P)         # [kh, p, rt, rl, c]
             .transpose(1, 2, 3, 0, 4)          # [p, rt, rl, kh, c]
             .reshape(P, 8 * RS * 2 * P))
        # b1 inject tile per rt: [rl, (rt, c)]
        b1c = (b1s[:, csl].reshape(8, RS, P)    # [rt, rl, c]
               .transpose(1, 0, 2).reshape(RS, 8 * P))
        cpack = np.concatenate(
            [embT.astype(f32), wihT.astype(f32), whhT.astype(f32),
             e2dT.astype(f32), eye, pad128(b1c), pad128(delta)], axis=1)
        in_maps.append(dict(feat=feat[2 * k:2 * k + 2], cpack=to_bf(cpack),
                            biasf=biasf, w1T=to_bf(a)))
    return in_maps, slots, T0


_cache = {}


def _run(inputs, trace=False):
    (in_maps, slots, T0) = _prep_inputs(
        inputs["feature"], inputs["instruction_idx"],
        inputs["instruction_length"], inputs["emb_table"],
        inputs["w_ih"], inputs["w_hh"], inputs["b_ih"], inputs["b_hh"],
        inputs["e2d_w"], inputs["e2d_b"], inputs["lin1_w"], inputs["lin1_b"],
        inputs["bn_gamma"], inputs["bn_beta"], inputs["bn_mean"],
        inputs["bn_var"])

    key = tuple(slots)
    if key not in _cache:
        _cache[key] = _build_program(slots)
    nc = _cache[key]

    kw = {}
    if trace:
        kw = dict(trace=True, trace_cores=list(range(N_CORES)))
    res = run_bass_kernel_spmd(nc, in_maps, list(range(N_CORES)), **kw)
    # per-core out: [128=(j,q), 128=(b,n,m,blk)]; sample = 2*core + b,
    # hw = n*512 + blk*32 + q, value = max over 32-row group j of r-tile m.
    parts = np.stack([np.asarray(res.results[i]["part_out"], np.float32)
                      for i in range(N_CORES)])      # [8, 128, 128]
    v = parts.reshape(N_CORES, 4, 32, BS, 2, 2, 16)  # [c, j, q, b, n, m, blk]
    v = v.max(axis=(1, 5))                           # [core, q, b, n, blk]
    v = v.transpose(0, 2, 3, 4, 1)                   # [core, b, n, blk, q]
    single = v.reshape(B, HW2)
    single = np.maximum(single, T0)
    out = np.clip(single, 0.0, 1.0).reshape(B, 32, 32).astype(np.float32)
    return out, res


def kernel(**inputs) -> np.ndarray:
    out, _ = _run(inputs, trace=False)
    return out


def kernel_traced(**inputs):
    out, res = _run(inputs, trace=True)
    return out, res
